# revision 1
# baseline (speedup 1.0000x reference)
"""Trainium2 Bass kernel for nn_DOAM (dense CNN attention module).

Strategy: pure data parallel (4 images/core x 8 cores). Convs are computed as
"row-batched banded GEMMs": for a group of R output rows, M = R*Cout output
partitions, K = (R+2)*Cin input partitions (rows interleaved row-major with
channels), and the 3 kernel-x taps are 3 matmuls accumulating in PSUM with
shifted rhs column windows.  All matmul operands fp16, PSUM fp32.

SBUF-resident panels keep the 2 halo rows at the END of the partition range
(interior rows at partition 0) so PSUM evacuations are base-partition-0;
the banded lhsT is row-permuted to match.  Halo rows are filled with small
SBUF->SBUF DMAs from neighbouring panels.

Three device phases:
  A1: conv1..conv5 -> x (to DRAM, fp16)
  host: 5/10/15 block-average pools of x + nearest upsample (0.3% of FLOPs)
  A2: c5/c10/c15 (concat convs), wg/wm gated conv, batch-norm partial sums
  host: BN statistics fold (into a per-channel scale/offset of gated)
  B : 8->1 conv + sigmoid + residual blend with the input image
"""
import sys
import numpy as np
from contextlib import ExitStack

sys.path.insert(0, "/opt/trn_rl_repo")
import concourse.bacc as bacc
import concourse.tile as tile
from concourse import mybir
from concourse.bass_utils import run_bass_kernel_spmd

F16 = mybir.dt.float16
F32 = mybir.dt.float32
AF = mybir.ActivationFunctionType
ALU = mybir.AluOpType

H = W = 300
HP = WP = 302
NCORES = 8
BPC = 4          # images per core
EPS = 1e-5

_NC_CACHE = {}


def _bacc():
    return bacc.Bacc("TRN2", target_bir_lowering=False, debug=False,
                     enable_asserts=True, num_devices=1)


def band_lhs(w, R, cin_idx, cout_idx, perm=False):
    """w [O,I,3,3] -> [K=(R+2)*len(cin), 3*M] fp32, M=R*len(cout).

    Window position p (0 = top halo, 1..R interior, R+1 = bottom halo) maps to
    partition row-block p (natural, DMA-fed panels) or, when perm=True,
    interior first: p in 1..R -> p-1, p==0 -> R, p==R+1 -> R+1."""
    Cb, Ob = len(cin_idx), len(cout_idx)
    K, M = (R + 2) * Cb, R * Ob
    lhs = np.zeros((3, K, M), np.float32)
    for dx in range(3):
        for yo in range(R):
            for dy in range(3):
                p = yo + dy
                blk = (R if p == 0 else (p - 1 if p <= R else p)) if perm else p
                for oi, o in enumerate(cout_idx):
                    for ci, c in enumerate(cin_idx):
                        lhs[dx, blk * Cb + ci, yo * Ob + oi] = w[o, c, dy, dx]
    return np.ascontiguousarray(lhs.transpose(1, 0, 2).reshape(K, 3 * M))


def tile_bias(b, R):
    return np.tile(np.asarray(b, np.float32), R)[:, None]  # [R*O, 1]


# --------------------------------------------------------------------------
# phase A1: conv1..conv5
# --------------------------------------------------------------------------

def build_a1():
    nc = _bacc()
    im16 = nc.dram_tensor("im16", [BPC, 3, HP, WP], F16, kind="ExternalInput").ap()
    wts, bia = {}, {}
    specs = {  # name -> (K, M)
        "l1": (24, 48), "l2": (64, 96),
        "l3a": (128, 96), "l3b": (128, 96),
        "l4a": (128, 96), "l4b": (128, 96),
        "l5": (128, 48),
    }
    for nm, (K, M) in specs.items():
        wts[nm] = nc.dram_tensor(f"w_{nm}", [K, 3 * M], F16, kind="ExternalInput").ap()
    for nm, M in [("l1", 48), ("l2", 96), ("l3a", 96), ("l3b", 96),
                  ("l4", 96), ("l5", 48)]:
        bia[nm] = nc.dram_tensor(f"b_{nm}", [M, 1], F32, kind="ExternalInput").ap()
    x16 = nc.dram_tensor("x16", [BPC, HP, 8, WP], F16, kind="ExternalOutput").ap()

    with tile.TileContext(nc) as tc, ExitStack() as ctx:
        wp = ctx.enter_context(tc.tile_pool(name="wp", bufs=1))
        W16, B32 = {}, {}
        for nm, (K, M) in specs.items():
            t = wp.tile([K, 3 * M], F16, tag=f"w{nm}")
            nc.sync.dma_start(t[:], wts[nm][:])
            W16[nm] = t
        for nm in bia:
            t = wp.tile([bia[nm].shape[0], 1], F32, tag=f"b{nm}")
            nc.sync.dma_start(t[:], bia[nm][:])
            B32[nm] = t
        zt = wp.tile([16, WP], F16, tag="zt")
        nc.vector.memset(zt[:], 0.0)

        p_im = ctx.enter_context(tc.tile_pool(name="p_im", bufs=4))
        p2 = ctx.enter_context(tc.tile_pool(name="p2", bufs=6))
        p3 = ctx.enter_context(tc.tile_pool(name="p3", bufs=6))
        p4a = ctx.enter_context(tc.tile_pool(name="p4a", bufs=6))
        p4b = ctx.enter_context(tc.tile_pool(name="p4b", bufs=6))
        p5 = ctx.enter_context(tc.tile_pool(name="p5", bufs=6))
        ps = ctx.enter_context(tc.tile_pool(name="ps", bufs=8, space="PSUM"))
        ev = ctx.enter_context(tc.tile_pool(name="ev", bufs=4))

        for img in range(BPC):
            P2, P3, P4A, P4B, P5 = {}, {}, {}, {}, {}

            def mm3(pt, wtile, K, M, pan, start=True, stop=True):
                for dx in range(3):
                    nc.tensor.matmul(pt[0:M, :], wtile[:K, dx * M:dx * M + M],
                                     pan[:K, dx:dx + W],
                                     start=(start and dx == 0),
                                     stop=(stop and dx == 2))

            def halo(panels, t, C):
                """fill halo row-blocks (R..R+2) of permuted panel t (R=6)."""
                pan = panels[t]
                if t == 0:
                    nc.sync.dma_start(pan[6 * C:7 * C, :], zt[:C, :])
                else:
                    nc.sync.dma_start(pan[6 * C:7 * C, :], panels[t - 1][5 * C:6 * C, :])
                if t == 49:
                    nc.sync.dma_start(pan[7 * C:8 * C, :], zt[:C, :])
                else:
                    nc.sync.dma_start(pan[7 * C:8 * C, :], panels[t + 1][0:C, :])

            def evac_dve(dst, n, pt, m, btile):
                nc.vector.tensor_scalar(dst[0:n, 1:301], pt[0:m, :], btile, None,
                                        op0=ALU.add)
                nc.vector.memset(dst[0:n, 0:1], 0.0)
                nc.vector.memset(dst[0:n, 301:302], 0.0)

            def evac_act(dst, n, pt, m, btile):
                nc.scalar.activation(dst[0:n, 1:301], pt[0:m, :], AF.Identity,
                                     bias=btile)
                nc.vector.memset(dst[0:n, 0:1], 0.0)
                nc.vector.memset(dst[0:n, 301:302], 0.0)

            def L1(t):  # R6, rows 6t..6t+5, im panel DMA-fed (natural order)
                pan = p_im.tile([24, WP], F16, tag="imp")
                nc.sync.dma_start(pan[:],
                                  im16[img, :, 6 * t:6 * t + 8, :]
                                  .rearrange("c y x -> y c x"))
                pt = ps.tile([48, W], F32, tag="ps")
                mm3(pt, W16["l1"], 24, 48, pan)
                dst = p2.tile([64, WP], F16, tag="p2")
                P2[t] = dst
                evac_dve(dst, 48, pt, 48, B32["l1"][:])

            def L2(t):
                halo(P2, t, 8)
                pt = ps.tile([96, W], F32, tag="ps")
                mm3(pt, W16["l2"], 64, 96, P2[t])
                dst = p3.tile([128, WP], F16, tag="p3")
                P3[t] = dst
                evac_act(dst, 96, pt, 96, B32["l2"][:])

            def L3(t):
                halo(P3, t, 16)
                pta = ps.tile([96, W], F32, tag="ps")
                ptb = ps.tile([96, W], F32, tag="ps")
                mm3(pta, W16["l3a"], 128, 96, P3[t])
                mm3(ptb, W16["l3b"], 128, 96, P3[t])
                for nm, pt, pool, store, ed in (("l3a", pta, p4a, P4A, evac_dve),
                                                ("l3b", ptb, p4b, P4B, evac_act)):
                    dst = pool.tile([128, WP], F16, tag=nm)
                    store[t] = dst
                    ed(dst, 96, pt, 96, B32[nm][:])

            def L4(t):
                halo(P4A, t, 16)
                halo(P4B, t, 16)
                pt = ps.tile([96, W], F32, tag="ps")
                for bi, (wnm, pan) in enumerate((("l4a", P4A[t]), ("l4b", P4B[t]))):
                    for dx in range(3):
                        nc.tensor.matmul(pt[:, :], W16[wnm][:, dx * 96:dx * 96 + 96],
                                         pan[:128, dx:dx + W],
                                         start=(bi == 0 and dx == 0),
                                         stop=(bi == 1 and dx == 2))
                dst = p5.tile([128, WP], F16, tag="p5")
                P5[t] = dst
                evac_dve(dst, 96, pt, 96, B32["l4"][:])

            def L5(t):
                halo(P5, t, 16)
                pt = ps.tile([48, W], F32, tag="ps")
                mm3(pt, W16["l5"], 128, 48, P5[t])
                o = ev.tile([48, W], F16, tag="xev")
                nc.vector.tensor_scalar(o[:, :], pt[:, :], B32["l5"][:], None,
                                        op0=ALU.add)
                nc.sync.dma_start(x16[img, 6 * t + 1:6 * t + 7, :, 1:301], o[:, :])

            for s in range(0, 58):
                if s < 50:
                    L1(s)
                if 0 <= s - 2 < 50:
                    L2(s - 2)
                if 0 <= s - 4 < 50:
                    L3(s - 4)
                if 0 <= s - 6 < 50:
                    L4(s - 6)
                if 0 <= s - 8 < 50:
                    L5(s - 8)
    nc.finalize()
    return nc


# --------------------------------------------------------------------------
# phase A2: c5/c10/c15, wg/wm, gated, BN partial sums
# --------------------------------------------------------------------------

def build_a2():
    nc = _bacc()
    x16 = nc.dram_tensor("x16", [BPC, HP, 8, WP], F16, kind="ExternalInput").ap()
    up = {k: nc.dram_tensor(f"up{k}", [BPC, HP, 8, WP], F16,
                            kind="ExternalInput").ap() for k in (5, 10, 15)}
    wts, bia = {}, {}
    for nm in ("c5x", "c5u", "c10x", "c10u", "c15x", "c15u",
               "wg0", "wg1", "wg2", "wm0", "wm1", "wm2"):
        wts[nm] = nc.dram_tensor(f"w_{nm}", [112, 3 * 96], F16,
                                 kind="ExternalInput").ap()
    for nm in ("c5", "c10", "c15", "wg", "wm"):
        bia[nm] = nc.dram_tensor(f"b_{nm}", [96, 1], F32, kind="ExternalInput").ap()
    gat = nc.dram_tensor("gat", [BPC, HP, 8, WP], F32, kind="ExternalOutput").ap()
    stats = nc.dram_tensor("stats", [BPC, 96, 2], F32, kind="ExternalOutput").ap()

    with tile.TileContext(nc) as tc, ExitStack() as ctx:
        wp = ctx.enter_context(tc.tile_pool(name="wp", bufs=1))
        W16, B32 = {}, {}
        for nm in wts:
            t = wp.tile([112, 3 * 96], F16, tag=f"w{nm}")
            nc.sync.dma_start(t[:], wts[nm][:])
            W16[nm] = t
        for nm in bia:
            t = wp.tile([96, 1], F32, tag=f"b{nm}")
            nc.sync.dma_start(t[:], bia[nm][:])
            B32[nm] = t
        zt = wp.tile([8, WP], F16, tag="zt")
        nc.vector.memset(zt[:], 0.0)

        pin = ctx.enter_context(tc.tile_pool(name="pin", bufs=3))
        pc = {k: ctx.enter_context(tc.tile_pool(name=f"pc{k}", bufs=6))
              for k in (5, 10, 15)}
        ps = ctx.enter_context(tc.tile_pool(name="ps", bufs=8, space="PSUM"))
        ev = ctx.enter_context(tc.tile_pool(name="ev", bufs=3))
        st = ctx.enter_context(tc.tile_pool(name="st", bufs=2))

        for img in range(BPC):
            CP = {5: {}, 10: {}, 15: {}}
            acc = st.tile([96, 2], F32, tag="acc")
            nc.vector.memset(acc[:], 0.0)

            def CL(k, t):  # c5/c10/c15 group t (R12), panels DMA-fed natural
                panx = pin.tile([112, WP], F16, tag="panx")
                nc.sync.dma_start(panx[:], x16[img, 12 * t:12 * t + 14, :, :])
                panu = pin.tile([112, WP], F16, tag=f"panu{k}")
                nc.sync.dma_start(panu[:], up[k][img, 12 * t:12 * t + 14, :, :])
                pt = ps.tile([96, W], F32, tag="ps")
                for bi, (wnm, pan) in enumerate(((f"c{k}x", panx), (f"c{k}u", panu))):
                    for dx in range(3):
                        nc.tensor.matmul(pt[:, :], W16[wnm][:, dx * 96:dx * 96 + 96],
                                         pan[:, dx:dx + W],
                                         start=(bi == 0 and dx == 0),
                                         stop=(bi == 1 and dx == 2))
                dst = pc[k].tile([112, WP], F16, tag=f"cp{k}")
                CP[k][t] = dst
                nc.vector.tensor_scalar(dst[0:96, 1:301], pt[:, :],
                                        B32[f"c{k}"][:], None, op0=ALU.add)
                nc.vector.memset(dst[0:96, 0:1], 0.0)
                nc.vector.memset(dst[0:96, 301:302], 0.0)

            def halo12(panels, t):  # permuted layout, C=8, R=12
                pan = panels[t]
                if t == 0:
                    nc.sync.dma_start(pan[96:104, :], zt[:, :])
                else:
                    nc.sync.dma_start(pan[96:104, :], panels[t - 1][88:96, :])
                if t == 24:
                    nc.sync.dma_start(pan[104:112, :], zt[:, :])
                else:
                    nc.sync.dma_start(pan[104:112, :], panels[t + 1][0:8, :])

            def GATED(t):
                for k in (5, 10, 15):
                    halo12(CP[k], t)
                ptg = ps.tile([96, W], F32, tag="ps")
                ptm = ps.tile([96, W], F32, tag="ps")
                for pt, pfx in ((ptg, "wg"), (ptm, "wm")):
                    for bi, k in enumerate((5, 10, 15)):
                        wtile = W16[f"{pfx}{bi}"]
                        for dx in range(3):
                            nc.tensor.matmul(pt[:, :],
                                             wtile[:, dx * 96:dx * 96 + 96],
                                             CP[k][t][:, dx:dx + W],
                                             start=(bi == 0 and dx == 0),
                                             stop=(bi == 2 and dx == 2))
                s = ev.tile([96, W], F32, tag="sig")
                nc.scalar.activation(s[:, :], ptm[:, :], AF.Sigmoid,
                                     bias=B32["wm"][:])
                g = ev.tile([96, W], F32, tag="gg")
                nc.vector.tensor_scalar(g[:, :], ptg[:, :], B32["wg"][:], None,
                                        op0=ALU.add)
                gv = ev.tile([96, W], F32, tag="gv")
                nc.vector.tensor_tensor(gv[:, :], g[:, :], s[:, :], op=ALU.mult)
                nc.sync.dma_start(gat[img, 12 * t + 1:12 * t + 13, :, 1:301], gv[:, :])
                red = ev.tile([96, 2], F32, tag="red")
                nc.vector.tensor_reduce(red[:, 0:1], gv[:, :],
                                        axis=mybir.AxisListType.X, op=ALU.add)
                sq = ev.tile([96, W], F32, tag="sq")
                nc.vector.tensor_tensor(sq[:, :], gv[:, :], gv[:, :], op=ALU.mult)
                nc.vector.tensor_reduce(red[:, 1:2], sq[:, :],
                                        axis=mybir.AxisListType.X, op=ALU.add)
                nc.vector.tensor_tensor(acc[:, :], acc[:, :], red[:, :], op=ALU.add)

            for u in range(0, 27):
                if u < 25:
                    for k in (5, 10, 15):
                        CL(k, u)
                if 0 <= u - 2 < 25:
                    GATED(u - 2)
            nc.sync.dma_start(stats[img, :, :], acc[:, :])
    nc.finalize()
    return nc


# --------------------------------------------------------------------------
# phase B: normalized-gated 8->1 conv, sigmoid, residual blend
# --------------------------------------------------------------------------

def build_b(gamma, b11v):
    nc = _bacc()
    gatd = nc.dram_tensor("gat", [BPC, HP, 8, WP], F32, kind="ExternalInput").ap()
    im32 = nc.dram_tensor("im32", [BPC, 3, HP, WP], F32, kind="ExternalInput").ap()
    w11d = nc.dram_tensor("w_l11", [112, 3 * 12], F16, kind="ExternalInput").ap()
    scd = nc.dram_tensor("sc", [112, 6], F32, kind="ExternalInput").ap()
    bcd = nc.dram_tensor("bcmat", [12, 36], F16, kind="ExternalInput").ap()
    outd = nc.dram_tensor("out", [BPC, H, 3, W], F32, kind="ExternalOutput").ap()

    with tile.TileContext(nc) as tc, ExitStack() as ctx:
        wp = ctx.enter_context(tc.tile_pool(name="wp", bufs=1))
        w11 = wp.tile([112, 3 * 12], F16, tag="w11")
        nc.sync.dma_start(w11[:], w11d[:])
        sc = wp.tile([112, 6], F32, tag="sc")
        nc.sync.dma_start(sc[:], scd[:])
        bc = wp.tile([12, 36], F16, tag="bc")
        nc.sync.dma_start(bc[:], bcd[:])
        cg = wp.tile([36, 1], F32, tag="cg")
        nc.vector.memset(cg[:], 1.0 - gamma)
        b11t = wp.tile([12, 1], F32, tag="b11t")
        nc.vector.memset(b11t[:], b11v)

        pin = ctx.enter_context(tc.tile_pool(name="pin", bufs=4))
        ps = ctx.enter_context(tc.tile_pool(name="ps", bufs=4, space="PSUM"))
        ev = ctx.enter_context(tc.tile_pool(name="ev", bufs=4))

        for img in range(BPC):
            for t in range(25):
                gp32 = pin.tile([112, WP], F32, tag="gp32")
                nc.sync.dma_start(gp32[:], gatd[img, 12 * t:12 * t + 14, :, :])
                gn = pin.tile([112, WP], F16, tag="gn")
                j = 1 if t == 0 else (2 if t == 24 else 0)
                nc.vector.tensor_scalar(gn[:, 1:301], gp32[:, 1:301],
                                        sc[:, 2 * j:2 * j + 1],
                                        sc[:, 2 * j + 1:2 * j + 2],
                                        op0=ALU.mult, op1=ALU.add)
                nc.vector.memset(gn[:, 0:1], 0.0)
                nc.vector.memset(gn[:, 301:302], 0.0)
                ptz = ps.tile([12, W], F32, tag="ps")
                for dx in range(3):
                    nc.tensor.matmul(ptz[:, :], w11[:, dx * 12:dx * 12 + 12],
                                     gn[:, dx:dx + W],
                                     start=(dx == 0), stop=(dx == 2))
                sg = ev.tile([12, W], F16, tag="sg")
                nc.scalar.activation(sg[:, :], ptz[:, :], AF.Sigmoid, bias=b11t[:])
                ptf = ps.tile([36, W], F32, tag="ps")
                nc.tensor.matmul(ptf[:, :], bc[:, :], sg[:, :], start=True, stop=True)
                f3 = ev.tile([36, W], F32, tag="f3")
                nc.scalar.activation(f3[:, :], ptf[:, :], AF.Identity,
                                     bias=cg[:], scale=float(gamma))
                imp = pin.tile([36, WP], F32, tag="imp")
                nc.sync.dma_start(imp[:], im32[img, :, 12 * t + 1:12 * t + 13, :]
                                  .rearrange("c y x -> y c x"))
                o = ev.tile([36, W], F32, tag="o")
                nc.vector.tensor_tensor(o[:, :], imp[:, 1:301], f3[:, :],
                                        op=ALU.mult)
                nc.sync.dma_start(outd[img, 12 * t:12 * t + 12, :, :], o[:, :])
    nc.finalize()
    return nc


# --------------------------------------------------------------------------
# host orchestration
# --------------------------------------------------------------------------

def _pad_imgs(a, dtype):
    B, C = a.shape[:2]
    p = np.zeros((B, C, HP, WP), dtype)
    p[:, :, 1:301, 1:301] = a
    return p


def _a1_inputs(im, w1, b1, w2, b2, w3, b3, w4, b4, w5, b5):
    base = {
        "w_l1": band_lhs(w1, 6, range(3), range(8)).astype(np.float16),
        "w_l2": band_lhs(w2, 6, range(8), range(16), perm=True).astype(np.float16),
        "w_l3a": band_lhs(w3, 6, range(16), range(16), perm=True).astype(np.float16),
        "w_l3b": band_lhs(w3, 6, range(16), range(16, 32), perm=True).astype(np.float16),
        "w_l4a": band_lhs(w4, 6, range(16), range(16), perm=True).astype(np.float16),
        "w_l4b": band_lhs(w4, 6, range(16, 32), range(16), perm=True).astype(np.float16),
        "w_l5": band_lhs(w5, 6, range(16), range(8), perm=True).astype(np.float16),
        "b_l1": tile_bias(b1, 6), "b_l2": tile_bias(b2, 6),
        "b_l3a": tile_bias(np.asarray(b3)[:16], 6),
        "b_l3b": tile_bias(np.asarray(b3)[16:], 6),
        "b_l4": tile_bias(b4, 6), "b_l5": tile_bias(b5, 6),
    }
    im16 = _pad_imgs(im, np.float16)
    maps = []
    for c in range(NCORES):
        m = dict(base)
        m["im16"] = im16[c * BPC:(c + 1) * BPC]
        maps.append(m)
    return maps


def _pool_up(x):
    B = x.shape[0]
    ups = {}
    for k in (5, 10, 15):
        p = x.reshape(B, 8, 300 // k, k, 300 // k, k).mean(axis=(3, 5))
        u = np.repeat(np.repeat(p, k, axis=2), k, axis=3)
        ups[k] = np.ascontiguousarray(
            _pad_imgs(u, np.float16).transpose(0, 2, 1, 3))  # [B,302,8,302]
    return ups


def _a2_inputs(x16_by_core, wc5, bc5, wc10, bc10, wc15, bc15, wg, bg, wm, bm):
    base = {}
    for k, wc, bcv in ((5, wc5, bc5), (10, wc10, bc10), (15, wc15, bc15)):
        base[f"w_c{k}x"] = band_lhs(wc, 12, range(0, 8), range(8)).astype(np.float16)
        base[f"w_c{k}u"] = band_lhs(wc, 12, range(8, 16), range(8)).astype(np.float16)
        base[f"b_c{k}"] = tile_bias(bcv, 12)
    for pfx, wv, bv in (("wg", wg, bg), ("wm", wm, bm)):
        for bi in range(3):
            base[f"w_{pfx}{bi}"] = band_lhs(
                wv, 12, range(8 * bi, 8 * bi + 8), range(8),
                perm=True).astype(np.float16)
        base[f"b_{pfx}"] = tile_bias(bv, 12)
    maps = []
    for c in range(NCORES):
        x = np.asarray(x16_by_core[c]).reshape(BPC, HP, 8, WP)
        xin = x[:, 1:301, :, 1:301].transpose(0, 2, 1, 3).astype(np.float32)
        ups = _pool_up(xin)
        m = dict(base)
        m["x16"] = x
        for k in (5, 10, 15):
            m[f"up{k}"] = ups[k]
        maps.append(m)
    return maps


def kernel(im, w1, b1, w2, b2, w3, b3, w4, b4, w5, b5,
           wc5, bc5, wc10, bc10, wc15, bc15,
           wg, bg, wm, bm, bn_w, bn_b, w11, b11, gamma):
    im = np.asarray(im, np.float32)
    args = [np.asarray(a, np.float32) for a in
            (w1, b1, w2, b2, w3, b3, w4, b4, w5, b5,
             wc5, bc5, wc10, bc10, wc15, bc15, wg, bg, wm, bm)]
    (w1, b1, w2, b2, w3, b3, w4, b4, w5, b5,
     wc5, bc5, wc10, bc10, wc15, bc15, wg, bg, wm, bm) = args
    gamma_v = float(np.asarray(gamma).reshape(-1)[0])
    b11v = float(np.asarray(b11).reshape(-1)[0])
    core_ids = list(range(NCORES))

    if "a1" not in _NC_CACHE:
        _NC_CACHE["a1"] = build_a1()
    r1 = run_bass_kernel_spmd(_NC_CACHE["a1"],
                              _a1_inputs(im, w1, b1, w2, b2, w3, b3, w4, b4,
                                         w5, b5), core_ids)
    x16_by_core = [r1.results[c]["x16"] for c in core_ids]

    if "a2" not in _NC_CACHE:
        _NC_CACHE["a2"] = build_a2()
    r2 = run_bass_kernel_spmd(_NC_CACHE["a2"],
                              _a2_inputs(x16_by_core, wc5, bc5, wc10, bc10,
                                         wc15, bc15, wg, bg, wm, bm), core_ids)

    # ---- host BN fold
    ch_sum = np.zeros(8, np.float64)
    ch_sq = np.zeros(8, np.float64)
    for c in core_ids:
        s = np.asarray(r2.results[c]["stats"], np.float64).reshape(BPC, 12, 8, 2)
        s = s.sum(axis=(0, 1))
        ch_sum += s[:, 0]
        ch_sq += s[:, 1]
    n = 32.0 * H * W
    mean = ch_sum / n
    var = ch_sq / n - mean ** 2
    scale = np.asarray(bn_w, np.float64) / np.sqrt(var + EPS)
    off = np.asarray(bn_b, np.float64) - mean * scale
    sc_t = np.zeros((112, 6), np.float32)
    sc_t[:, 0] = np.tile(scale.astype(np.float32), 14)
    sc_t[:, 1] = np.tile(off.astype(np.float32), 14)
    sc_t[:, 2:4] = sc_t[:, 0:2]
    sc_t[0:8, 2:4] = 0.0        # t=0: top halo row is zero padding
    sc_t[:, 4:6] = sc_t[:, 0:2]
    sc_t[104:112, 4:6] = 0.0    # t=24: bottom halo row is zero padding
    bc_mat = np.zeros((12, 36), np.float16)
    for y in range(12):
        for ci in range(3):
            bc_mat[y, y * 3 + ci] = 1.0

    key_b = (gamma_v, b11v)
    if _NC_CACHE.get("b_key") != key_b:
        _NC_CACHE["b"] = build_b(gamma_v, b11v)
        _NC_CACHE["b_key"] = key_b
    w11b = band_lhs(np.asarray(w11, np.float32), 12, range(8),
                    range(1)).astype(np.float16)
    im32 = _pad_imgs(im, np.float32)
    maps = []
    for c in core_ids:
        maps.append({"gat": np.asarray(r2.results[c]["gat"]).reshape(BPC, HP, 8, WP),
                     "im32": im32[c * BPC:(c + 1) * BPC],
                     "w_l11": w11b, "sc": sc_t, "bcmat": bc_mat})
    r3 = run_bass_kernel_spmd(_NC_CACHE["b"], maps, core_ids)

    out = np.empty((32, 3, H, W), np.float32)
    for c in core_ids:
        o = np.asarray(r3.results[c]["out"]).reshape(BPC, H, 3, W)
        out[c * BPC:(c + 1) * BPC] = o.transpose(0, 2, 1, 3)
    return out

